# revision 55
# baseline (speedup 1.0000x reference)
"""Trainium2 Bass kernel for 16-head causal MultiHeadAttention.

Problem: x [4, 2048, 1024], 16 heads of dim 64, causal softmax attention,
output projection Wo [1024, 1024] + bo.

Sharding over 8 NeuronCores: core c handles batch b = c // 2 and head-group
g = c % 2 (8 heads each).  Each core computes its 8 heads' Q/K/V projections,
causal attention, and a partial output projection against its row-slice of
Wo.  The two cores of a batch return partial [D, S] fp16 outputs that the
host sums, transposes, and biases.

On-core design (v2 — fp8 DoubleRow):
  - x and the Q/K/V weights are fp8(e4m3); weights/biases are pre-scaled by
    32 on the host so their magnitudes sit in e4m3's sweet spot.  Projections
    run as DoubleRow chains (K=256 per pass, half the PE passes of fp16);
    Q^T/K^T land in fp16 carrying the x32 scale, which is folded into the
    softmax exp scale (0.125/32^2).
  - Heads are processed in pairs (2 x 64 = 128 partitions).  Scores are
    computed transposed, ST[t, s] = K @ Q^T, one 128-row t-tile at a time,
    both heads written side by side into one 2-bank PSUM tile (the two
    64-contraction matmuls target disjoint PE row groups and disjoint PSUM
    banks so they can run concurrently).
  - One ScalarE exp per t-tile covers both heads via a strided AP that also
    trims the below-causal-frontier columns.  exp output is fp8: P in [0,1]
    and the softmax renormalization forgives the 2% quantization.
  - P is contracted with V' = [32V | 1] (fp8) so each AV matmul also
    accumulates the softmax denominator in PSUM rows 64..127.  Full
    (below-diagonal) t-tiles go through DoubleRow AV in pairs; diagonal
    tiles run as single fp8 matmuls with causal column trimming plus one
    multiplicative triangular mask per head.
  - V is transposed to natural [t, dk] layout with PE transposes; a single
    strided DVE copy per tile drops both heads' slices into V' (fp8).
  - Output projection stays fp16 (fp8 would put ~3% noise directly on the
    output): per j-block so it can overlap the last pair's attention,
    accumulated over the 4 pairs in PSUM, written out as fp16 (the host
    sums the two half-partials in fp32).
  - All inputs arrive host-pre-arranged so every DMA is contiguous, ordered
    by first use (pair-0 fp16 block-0 set, then x fp8, then fp8 weights);
    per-pair weight blocks stay contiguous in the consolidated transfers so
    DoubleRow LDWEIGHTS reads 256B runs (strided weight slices disable FWL
    and cost ~25%% matmul throughput).

Measured: 243.0us (baseline 329.8us), rel err 3.2e-3 vs the 2e-2 gate.
"""

import sys

for _p in ("/opt/trn_rl_repo", "/root/.axon_site/_ro/trn_rl_repo"):
    if _p not in sys.path:
        sys.path.insert(0, _p)

import ml_dtypes
import numpy as np

import concourse.bacc as bacc
import concourse.mybir as mybir
from concourse import bass_utils
from concourse.masks import make_identity, make_upper_triangular
from concourse.tile import TileContext

P = 128
S = 2048  # sequence length
D = 1024  # hidden size
H = 16  # total heads
DK = 64  # head dim
B = 4  # batch
NCORES = 8
HPC = 8  # heads per core
NPAIR = HPC // 2  # head pairs per core
SB = 512  # s-block width
NSB = S // SB  # 4
TT = S // P  # 16 t-tiles
DT = D // P  # 8 d-tiles
VW = 2 * DK  # V' width per t-tile (64 V columns | 64 ones columns)

WSCALE = 32.0  # host-side scale on Wq/Wk/Wv/biases (fp8 dynamic range)
SCL = 0.125 / (WSCALE * WSCALE)  # exp scale: 1/sqrt(DK) / (32*32)

F32 = mybir.dt.float32
F16 = mybir.dt.float16
F8 = mybir.dt.float8e4
AF = mybir.ActivationFunctionType
MUL = mybir.AluOpType.mult
DR = mybir.MatmulPerfMode.DoubleRow


def build_nc(debug=False):
    nc = bacc.Bacc()
    # all inputs arrive pre-arranged on the host so every DMA is a plain
    # contiguous transfer: x as [r, (d s)], weights as [r, (p d c)]
    xT = nc.dram_tensor("xT", [P, DT * S], F8, kind="ExternalInput")
    # fp16 copies for the first 128 sequence positions' projections: fp8
    # noise there lands on rows whose softmax averages over too few keys to
    # forgive it.  xT8j0 duplicates x's block-0 columns so those DoubleRow
    # chains don't wait on the big x transfer at startup.
    XE = 128  # fp16-exact prefix width
    # boot = per-d [wq16 pair-0 block | x block-0 fp16]: the two tensors
    # that gate the very first matmul chain arrive in ONE transfer
    boot = nc.dram_tensor("boot", [P, DT * (P + XE)], F16, kind="ExternalInput")
    xT8j0 = nc.dram_tensor("xT8j0", [P, DT * SB], F8, kind="ExternalInput")
    wq16 = nc.dram_tensor("wq16", [P, NPAIR * DT * P], F16, kind="ExternalInput")
    wk16 = nc.dram_tensor("wk16", [P, NPAIR * DT * P], F16, kind="ExternalInput")
    wv16 = nc.dram_tensor("wv16", [P, NPAIR * DT * P], F16, kind="ExternalInput")
    wq = nc.dram_tensor("wq", [P, NPAIR * DT * P], F8, kind="ExternalInput")
    wk = nc.dram_tensor("wk", [P, NPAIR * DT * P], F8, kind="ExternalInput")
    wv = nc.dram_tensor("wv", [P, NPAIR * DT * P], F8, kind="ExternalInput")
    wo_t = nc.dram_tensor("wo_t", [HPC * DK, D], F16, kind="ExternalInput")
    # q|k|v biases in one transfer
    bqkv = nc.dram_tensor("bqkv", [P, 3 * NPAIR], F32, kind="ExternalInput")
    out = nc.dram_tensor("out_part", [D, S], F16, kind="ExternalOutput")
    dbg = {}
    if debug:
        for nm, shp, dt_ in (
            ("dbg_qt", [P, S], F16),
            ("dbg_kt", [P, S], F16),
            ("dbg_vp", [P, 2 * TT * VW], F16),
            ("dbg_ot", [P, S], F16),
        ):
            dbg[nm] = nc.dram_tensor(nm, shp, dt_, kind="ExternalOutput")

    with TileContext(nc) as tc:
        from contextlib import ExitStack

        with ExitStack() as ctx:
            pool = lambda *a, **k: ctx.enter_context(tc.tile_pool(*a, **k))
            const_pool = pool(name="const", bufs=1)
            xt_pool = pool(name="xt", bufs=1)
            xt16_pool = pool(name="xt16", bufs=1)
            wgt_pool = pool(name="wgt", bufs=3 * NPAIR)
            wo_pool = pool(name="wo", bufs=NPAIR)
            qt_pool = pool(name="qt", bufs=2)
            kt_pool = pool(name="kt", bufs=2)
            vp_pool = pool(name="vp", bufs=2)
            vstg_pool = pool(name="vstg", bufs=2)
            wt_pool = pool(name="wt", bufs=3)
            ot_pool = pool(name="ot", bufs=NPAIR)
            rcs_pool = pool(name="rcs", bufs=3)
            ost_pool = pool(name="ost", bufs=4)
            ps_sc = pool(name="ps_sc", bufs=2, space="PSUM")
            ps_pa = pool(name="ps_pa", bufs=2, space="PSUM")
            ps_pv = pool(name="ps_pv", bufs=2, space="PSUM")

            # --- constants ---
            ident = const_pool.tile([P, P], F16)
            make_identity(nc, ident[:])
            # mask_ut[r, c] = 1 if c >= r else 0 (causal boundary block)
            mask_ut = const_pool.tile([P, P], F16)
            make_upper_triangular(nc, mask_ut[:], val=1.0, diag=True)

            # --- resident inputs, in first-use order: the pair-0 block-0
            # fp16 inputs gate the very first matmul chain, so they go
            # first and everything else queues behind them ---
            def load_wgt16(srcw, p, name):
                t = wgt_pool.tile([P, DT, P], F16, tag="wgt", name=name)
                nc.sync.dma_start(
                    t[:], srcw[:, p * DT * P : (p + 1) * DT * P]
                )
                return t

            boot_t = xt16_pool.tile([P, DT, P + XE], F16, tag="boot", name="boot")
            nc.sync.dma_start(boot_t[:], boot[:])
            wtiles16 = [{"q": None}]  # pair-0 Q comes from boot_t
            xt8j0 = xt16_pool.tile([P, DT, SB], F8, tag="xt8j0", name="xt8j0")
            nc.sync.dma_start(xt8j0[:], xT8j0[:])
            # fp8 weights: pair-0 slice first (gates the first DoubleRow
            # chains), remaining pairs later; per-pair [d, c] blocks stay
            # contiguous so DoubleRow LDWEIGHTS reads 256B runs (FWL)
            w8_all = {}
            w8_src = {"q": wq, "k": wk, "v": wv}
            for nm in ("q", "k", "v"):
                w8_all[nm] = wgt_pool.tile(
                    [P, NPAIR, DT, P], F8, tag="wgt", name=f"w8{nm}"
                )
            nc.sync.dma_start(w8_all["q"][:, 0, :, :], wq[:, : DT * P])
            wtiles16[0]["k"] = load_wgt16(wk16, 0, "wk16_0")
            nc.sync.dma_start(w8_all["k"][:, 0, :, :], wk[:, : DT * P])
            wtiles16[0]["v"] = load_wgt16(wv16, 0, "wv16_0")
            nc.sync.dma_start(w8_all["v"][:, 0, :, :], wv[:, : DT * P])
            bqkv_t = const_pool.tile([P, 3 * NPAIR], F32)
            nc.sync.dma_start(bqkv_t[:], bqkv[:])
            boff = {"q": 0, "k": NPAIR, "v": 2 * NPAIR}
            xt = xt_pool.tile([P, DT, S], F8, tag="xt", name="xt")
            for dh in range(2):
                cs = slice(dh * (DT // 2), (dh + 1) * (DT // 2))
                nc.sync.dma_start(
                    xt[:, cs, :], xT[:, dh * DT * S // 2 : (dh + 1) * DT * S // 2]
                )
            for nm in ("q", "k", "v"):
                nc.sync.dma_start(
                    w8_all[nm][:, 1:NPAIR, :, :], w8_src[nm][:, DT * P :]
                )
            for p in range(1, NPAIR):
                wtiles16.append(
                    {
                        nm: load_wgt16(srcw, p, f"w{nm}16_{p}")
                        for nm, srcw in (
                            ("q", wq16),
                            ("k", wk16),
                            ("v", wv16),
                        )
                    }
                )
            wo_tiles = []
            for p in range(NPAIR):
                t = wo_pool.tile([P, D], F16, tag="wo", name=f"wo{p}")
                nc.sync.dma_start(t[:], wo_t[p * P : (p + 1) * P, :])
                wo_tiles.append(t)

            # V' double buffers with the ones columns pre-set (the V copies
            # only ever touch columns 0..63 of each [tile, head] slot).
            # vp16 holds fp16 copies of tiles 0..3 for the strip-0 AV.
            vp_bufs, vp16_bufs = [], []
            for vb in range(2):
                t = vp_pool.tile([P, 2 * TT * VW], F8, tag="vp", name=f"vp{vb}")
                v4 = t[:].rearrange("r (h i c) -> r h i c", h=2, i=TT)
                t16 = vp_pool.tile(
                    [P, 2 * 4 * VW], F16, tag="vp16", name=f"vp16_{vb}"
                )
                v416 = t16[:].rearrange("r (h i c) -> r h i c", h=2, i=4)
                for h in range(2):
                    nc.vector.memset(v4[:, h, :, DK:VW], 1.0)
                    nc.vector.memset(v416[:, h, :, DK:VW], 1.0)
                vp_bufs.append(t)
                vp16_bufs.append(t16)

            ot_tiles = []
            for p in range(NPAIR):
                wts16 = wtiles16[p]
                qt = qt_pool.tile([P, S], F16, tag="qt")
                kt = kt_pool.tile([P, S], F16, tag="kt")
                vp = vp_bufs[p % 2]
                v4 = vp[:].rearrange("r (h i c) -> r h i c", h=2, i=TT)
                v416 = vp16_bufs[p % 2][:].rearrange(
                    "r (h i c) -> r h i c", h=2, i=4
                )
                ot = ot_pool.tile([P, S], F16, tag="ot")
                ot_tiles.append(ot)

                def proj(nm, j, ps):
                    # first XE columns in fp16 (accuracy for short-context
                    # rows), the rest as fp8 DoubleRow chains (half the PE
                    # passes); block 0 reads the early-arriving x copies
                    if j == 0:
                        for d in range(DT):
                            lhs16 = (
                                boot_t[:, d, 0:P]
                                if (p == 0 and nm == "q")
                                else wts16[nm][:, d, :]
                            )
                            nc.tensor.matmul(
                                ps[:, 0:XE],
                                lhs16,
                                boot_t[:, d, P : P + XE],
                                start=(d == 0),
                                stop=(d == DT - 1),
                            )
                        for dd in range(0, DT, 2):
                            nc.tensor.matmul(
                                ps[:, XE:SB],
                                w8_all[nm][:, p, dd : dd + 2, :],
                                xt8j0[:, dd : dd + 2, XE:SB],
                                start=(dd == 0),
                                stop=(dd == DT - 2),
                                perf_mode=DR,
                            )
                    else:
                        ss = slice(j * SB, (j + 1) * SB)
                        for dd in range(0, DT, 2):
                            nc.tensor.matmul(
                                ps[:],
                                w8_all[nm][:, p, dd : dd + 2, :],
                                xt[:, dd : dd + 2, ss],
                                start=(dd == 0),
                                stop=(dd == DT - 2),
                                perf_mode=DR,
                            )

                for j in range(NSB):
                    ss = slice(j * SB, (j + 1) * SB)
                    # --- Q/K projections (transposed [dk_pair, s] layout) ---
                    for nm, dest in (("q", qt), ("k", kt)):
                        ps = ps_pv.tile([P, SB], F32, tag="pv", name="ps_p")
                        proj(nm, j, ps)
                        nc.vector.tensor_scalar_add(
                            dest[:, ss],
                            ps[:],
                            bqkv_t[:, boff[nm] + p : boff[nm] + p + 1],
                        )

                    # --- V projection + PE transpose to natural [t, dk] ---
                    ps = ps_pv.tile([P, SB], F32, tag="pv", name="ps_v")
                    proj("v", j, ps)
                    vst = vstg_pool.tile([P, SB], F16, tag="vstg")
                    nc.vector.tensor_scalar_add(
                        vst[:],
                        ps[:],
                        bqkv_t[:, boff["v"] + p : boff["v"] + p + 1],
                    )
                    for u in range(SB // P):
                        tg = (SB // P) * j + u
                        pt = ps_pv.tile([P, P], F16, tag="pv", name="pt")
                        nc.tensor.transpose(
                            pt[:], vst[:, u * P : (u + 1) * P], ident[:]
                        )
                        # pt cols 0:64 = head0 V rows, 64:128 = head1; one
                        # strided copy drops both into V' (fp16 -> fp8)
                        nc.vector.tensor_copy(
                            v4[:, :, tg, 0:DK],
                            pt[:].rearrange("r (h c) -> r h c", h=2),
                        )
                        if j == 0:
                            nc.vector.tensor_copy(
                                v416[:, :, tg, 0:DK],
                                pt[:].rearrange("r (h c) -> r h c", h=2),
                            )

                    # --- causal attention strip j ---
                    pa0 = ps_pa.tile([P, SB], F32, tag="pa", name="pa0")
                    pa1 = ps_pa.tile([P, SB], F32, tag="pa", name="pa1")
                    nt = 4 * j + 4
                    wtp = None
                    for i in range(nt):
                        r = i - 4 * j
                        c0 = P * max(r, 0)
                        # both heads' scores into one PSUM tile: disjoint PE
                        # row groups and disjoint PSUM banks -> concurrent
                        sc = ps_sc.tile([P, 2 * SB], F32, tag="sc", name="sc")
                        nc.tensor.matmul(
                            sc[:, c0:SB],
                            kt[0:DK, i * P : (i + 1) * P],
                            qt[0:DK, j * SB + c0 : (j + 1) * SB],
                            start=True,
                            stop=True,
                        )
                        nc.tensor.matmul(
                            sc[:, SB + c0 : 2 * SB],
                            kt[DK:P, i * P : (i + 1) * P],
                            qt[DK:P, j * SB + c0 : (j + 1) * SB],
                            start=True,
                            stop=True,
                        )
                        if i % 2 == 0:
                            wtp = wt_pool.tile(
                                [P, 2 * 2 * SB], F16 if j == 0 else F8, tag="wt"
                            )
                        w4 = wtp[:].rearrange(
                            "r (t h c) -> r t h c", t=2, h=2
                        )
                        # one exp for both heads, trimmed to >= the causal
                        # frontier (strided AP over the two 512-col halves)
                        nc.scalar.activation(
                            w4[:, i % 2, :, c0:SB],
                            sc[:].rearrange("r (h c) -> r h c", h=2)[
                                :, :, c0:SB
                            ],
                            AF.Exp,
                            scale=SCL,
                        )
                        if r >= 0:
                            for h in range(2):
                                # on GpSimd: DVE is the busier engine and
                                # its backlog was stalling the V-transposes
                                nc.gpsimd.tensor_tensor(
                                    w4[:, i % 2, h, c0 : c0 + P],
                                    w4[:, i % 2, h, c0 : c0 + P],
                                    mask_ut[:],
                                    MUL,
                                )
                            if j == 0:
                                # strip 0: single fp16 AV matmul per tile
                                for h, pa in ((0, pa0), (1, pa1)):
                                    nc.tensor.matmul(
                                        pa[:, c0:],
                                        v416[:, h, i, :],
                                        w4[:, i % 2, h, c0:SB],
                                        start=(i == 0),
                                        stop=(i == nt - 1),
                                    )
                            elif r % 2 == 0:
                                # even diagonal tile: only its solo 128-col
                                # strip (the rest rides the next tile's DR)
                                for h, pa in ((0, pa0), (1, pa1)):
                                    nc.tensor.matmul(
                                        pa[:, c0 : c0 + P],
                                        v4[:, h, i, :],
                                        w4[:, i % 2, h, c0 : c0 + P],
                                        start=False,
                                        stop=False,
                                    )
                            else:
                                # odd diagonal tile: DoubleRow over the
                                # columns where both tiles are causally live
                                for h, pa in ((0, pa0), (1, pa1)):
                                    nc.tensor.matmul(
                                        pa[:, c0:],
                                        v4[:, h, i - 1 : i + 1, :],
                                        w4[:, :, h, c0:SB],
                                        start=False,
                                        stop=(i == nt - 1),
                                        perf_mode=DR,
                                    )
                        elif i % 2 == 1:
                            # two full t-tiles: DoubleRow AV (K=256)
                            for h, pa in ((0, pa0), (1, pa1)):
                                nc.tensor.matmul(
                                    pa[:],
                                    v4[:, h, i - 1 : i + 1, :],
                                    w4[:, :, h, :],
                                    start=(i == 1),
                                    stop=False,
                                    perf_mode=DR,
                                )
                    # normalize by the softmax denominator, which the
                    # ones-block of V' replicated into PSUM rows 64..127.
                    # (copy to SBUF first: reciprocal_approx_fast is a custom
                    # DVE op and cannot read PSUM)
                    den = rcs_pool.tile([P, SB], F32, tag="den", name="den")
                    nc.vector.tensor_copy(den[0:DK, :], pa0[DK:P, :])
                    nc.vector.tensor_copy(den[DK:P, :], pa1[DK:P, :])
                    rcs = rcs_pool.tile([P, SB], F32, tag="rcs", name="rcs")
                    nc.vector.reciprocal_approx_fast(rcs[:], den[:])
                    nc.vector.tensor_tensor(
                        ot[0:DK, ss], pa0[0:DK, :], rcs[0:DK, :], MUL
                    )
                    nc.vector.tensor_tensor(
                        ot[DK:P, ss], pa1[0:DK, :], rcs[DK:P, :], MUL
                    )
                if debug and p == 0:
                    nc.sync.dma_start(dbg["dbg_qt"][:], qt[:])
                    nc.sync.dma_start(dbg["dbg_kt"][:], kt[:])
                    vps = const_pool.tile([P, 2 * TT * VW], F16, name="vps")
                    nc.vector.tensor_copy(vps[:], vp[:])
                    nc.sync.dma_start(dbg["dbg_vp"][:], vps[:])
                    nc.sync.dma_start(dbg["dbg_ot"][:], ot[:])

            # --- output projection: j-major so block j overlaps the last
            # pair's attention on later blocks; accumulate the 4 pairs ---
            # (NB: tc.high_priority() here regressed — unbounded by 23us,
            # a bounded offset=500 on j=0 by ~3us.  Priority hoisting puts
            # sem-waits earlier in the runtime FIFO queues than their
            # dependencies resolve.)
            for j in range(NSB):
                ss = slice(j * SB, (j + 1) * SB)
                for m in range(DT):
                    ps = ps_pv.tile([P, SB], F32, tag="pv", name="ps_o")
                    for p in range(NPAIR):
                        nc.tensor.matmul(
                            ps[:],
                            wo_tiles[p][:, m * P : (m + 1) * P],
                            ot_tiles[p][:, ss],
                            start=(p == 0),
                            stop=(p == NPAIR - 1),
                        )
                    st = ost_pool.tile([P, SB], F16, tag="ost")
                    nc.vector.tensor_scalar_mul(st[:], ps[:], 1.0 / WSCALE)
                    nc.sync.dma_start(out[m * P : (m + 1) * P, ss], st[:])

    nc.compile()
    return nc


_NC_CACHE = None


def _get_nc():
    global _NC_CACHE
    if _NC_CACHE is None:
        _NC_CACHE = build_nc()
    return _NC_CACHE


def _f8(a):
    return np.asarray(a, dtype=np.float32).astype(ml_dtypes.float8_e4m3fn)


def _core_inputs(x, Wq, bq, Wk, bk, Wv, bv, Wo, c):
    b, g = c // 2, c % 2
    hs = slice(g * HPC, (g + 1) * HPC)
    heads = range(g * HPC, (g + 1) * HPC)
    def _xlay(a):
        # [D, ncol] -> [r, (d c)]
        return np.ascontiguousarray(
            a.reshape(DT, P, -1).transpose(1, 0, 2).reshape(P, -1)
        )

    def _wlay(a):
        # [D, HPC*DK] -> [r, (p d c)]
        return np.ascontiguousarray(
            a.reshape(DT, P, NPAIR, P).transpose(1, 2, 0, 3).reshape(P, -1)
        )

    xTf = np.ascontiguousarray(x[b].T)
    xT = _f8(_xlay(xTf))
    xT8j0 = _f8(_xlay(xTf[:, :SB]))
    wq_f = _wlay(WSCALE * np.concatenate([Wq[h] for h in heads], axis=1))
    wk_f = _wlay(WSCALE * np.concatenate([Wk[h] for h in heads], axis=1))
    wv_f = _wlay(WSCALE * np.concatenate([Wv[h] for h in heads], axis=1))
    wq_c, wk_c, wv_c = _f8(wq_f), _f8(wk_f), _f8(wv_f)
    boot_c = np.ascontiguousarray(
        np.concatenate(
            [
                wq_f[:, : DT * P].astype(np.float16).reshape(P, DT, P),
                _xlay(xTf[:, :128]).astype(np.float16).reshape(P, DT, 128),
            ],
            axis=2,
        ).reshape(P, -1)
    )
    def _blay(b_):
        return WSCALE * np.concatenate([b_[h] for h in heads]).reshape(NPAIR, P).T

    bqkv_c = np.ascontiguousarray(
        np.concatenate([_blay(bq), _blay(bk), _blay(bv)], axis=1),
        dtype=np.float32,
    )
    wo_c = np.ascontiguousarray(
        Wo[:, g * HPC * DK : (g + 1) * HPC * DK].T, dtype=np.float16
    )
    return {
        "xT": xT,
        "boot": boot_c,
        "xT8j0": xT8j0,
        "wq": wq_c,
        "wk": wk_c,
        "wv": wv_c,
        "wq16": wq_f.astype(np.float16),
        "wk16": wk_f.astype(np.float16),
        "wv16": wv_f.astype(np.float16),
        "wo_t": wo_c,
        "bqkv": bqkv_c,
    }


def kernel(x, Wq, bq, Wk, bk, Wv, bv, Wo, bo, _trace=False, _tmpdir=None):
    x = np.asarray(x, dtype=np.float32)
    nc = _get_nc()
    in_maps = [
        _core_inputs(x, Wq, bq, Wk, bk, Wv, bv, Wo, c) for c in range(NCORES)
    ]
    kw = {}
    if _trace:
        kw = dict(trace=True, tmpdir=_tmpdir)
    res = bass_utils.run_bass_kernel_spmd(
        nc, in_maps, core_ids=list(range(NCORES)), **kw
    )
    bo = np.asarray(bo, dtype=np.float32)
    out = np.empty((B, S, D), dtype=np.float32)
    for b in range(B):
        part = res.results[2 * b]["out_part"].astype(np.float32) + res.results[
            2 * b + 1
        ]["out_part"].astype(np.float32)
        out[b] = part.T + bo
    if _trace:
        kernel._last_results = res
    return out


# revision 56
# speedup vs baseline: 1.0232x; 1.0232x over previous
"""Trainium2 Bass kernel for 16-head causal MultiHeadAttention.

Problem: x [4, 2048, 1024], 16 heads of dim 64, causal softmax attention,
output projection Wo [1024, 1024] + bo.

Sharding over 8 NeuronCores: core c handles batch b = c // 2 and head-group
g = c % 2 (8 heads each).  Each core computes its 8 heads' Q/K/V projections,
causal attention, and a partial output projection against its row-slice of
Wo.  The two cores of a batch return partial [D, S] fp16 outputs that the
host sums, transposes, and biases.

On-core design (v2 — fp8 DoubleRow):
  - x and the Q/K/V weights are fp8(e4m3); weights/biases are pre-scaled by
    32 on the host so their magnitudes sit in e4m3's sweet spot.  Projections
    run as DoubleRow chains (K=256 per pass, half the PE passes of fp16);
    Q^T/K^T land in fp16 carrying the x32 scale, which is folded into the
    softmax exp scale (0.125/32^2).
  - Heads are processed in pairs (2 x 64 = 128 partitions).  Scores are
    computed transposed, ST[t, s] = K @ Q^T, one 128-row t-tile at a time,
    both heads written side by side into one 2-bank PSUM tile (the two
    64-contraction matmuls target disjoint PE row groups and disjoint PSUM
    banks so they can run concurrently).
  - One ScalarE exp per t-tile covers both heads via a strided AP that also
    trims the below-causal-frontier columns.  exp output is fp8: P in [0,1]
    and the softmax renormalization forgives the 2% quantization.
  - P is contracted with V' = [32V | 1] (fp8) so each AV matmul also
    accumulates the softmax denominator in PSUM rows 64..127.  Full
    (below-diagonal) t-tiles go through DoubleRow AV in pairs; diagonal
    tiles run as single fp8 matmuls with causal column trimming plus one
    multiplicative triangular mask per head.
  - V is transposed to natural [t, dk] layout with PE transposes; a single
    strided DVE copy per tile drops both heads' slices into V' (fp8).
  - Output projection stays fp16 (fp8 would put ~3% noise directly on the
    output): per j-block so it can overlap the last pair's attention,
    accumulated over the 4 pairs in PSUM, written out as fp16 (the host
    sums the two half-partials in fp32).
  - All inputs arrive host-pre-arranged so every DMA is contiguous, ordered
    by first use (pair-0 fp16 block-0 set, then x fp8, then fp8 weights);
    per-pair weight blocks stay contiguous in the consolidated transfers so
    DoubleRow LDWEIGHTS reads 256B runs (strided weight slices disable FWL
    and cost ~25%% matmul throughput).

Measured: 243.0us (baseline 329.8us), rel err 3.2e-3 vs the 2e-2 gate.
"""

import sys

for _p in ("/opt/trn_rl_repo", "/root/.axon_site/_ro/trn_rl_repo"):
    if _p not in sys.path:
        sys.path.insert(0, _p)

import ml_dtypes
import numpy as np

import concourse.bacc as bacc
import concourse.mybir as mybir
from concourse import bass_utils
from concourse.masks import make_identity, make_upper_triangular
from concourse.tile import TileContext

P = 128
S = 2048  # sequence length
D = 1024  # hidden size
H = 16  # total heads
DK = 64  # head dim
B = 4  # batch
NCORES = 8
HPC = 8  # heads per core
NPAIR = HPC // 2  # head pairs per core
SB = 512  # s-block width
NSB = S // SB  # 4
TT = S // P  # 16 t-tiles
DT = D // P  # 8 d-tiles
VW = 2 * DK  # V' width per t-tile (64 V columns | 64 ones columns)

WSCALE = 32.0  # host-side scale on Wq/Wk/Wv/biases (fp8 dynamic range)
SCL = 0.125 / (WSCALE * WSCALE)  # exp scale: 1/sqrt(DK) / (32*32)

F32 = mybir.dt.float32
F16 = mybir.dt.float16
F8 = mybir.dt.float8e4
AF = mybir.ActivationFunctionType
MUL = mybir.AluOpType.mult
DR = mybir.MatmulPerfMode.DoubleRow


def build_nc(debug=False):
    nc = bacc.Bacc()
    # all inputs arrive pre-arranged on the host so every DMA is a plain
    # contiguous transfer: x as [r, (d s)], weights as [r, (p d c)]
    xT = nc.dram_tensor("xT", [P, DT * S], F8, kind="ExternalInput")
    # fp16 copies for the first 128 sequence positions' projections: fp8
    # noise there lands on rows whose softmax averages over too few keys to
    # forgive it.  xT8j0 duplicates x's block-0 columns so those DoubleRow
    # chains don't wait on the big x transfer at startup.
    XE = 128  # fp16-exact prefix width
    # boot = per-d [wq16 pair-0 block | x block-0 fp16]: the two tensors
    # that gate the very first matmul chain arrive in ONE transfer
    boot = nc.dram_tensor("boot", [P, DT * (P + XE)], F16, kind="ExternalInput")
    xT8j0 = nc.dram_tensor("xT8j0", [P, DT * SB], F8, kind="ExternalInput")
    wq16 = nc.dram_tensor("wq16", [P, NPAIR * DT * P], F16, kind="ExternalInput")
    wk16 = nc.dram_tensor("wk16", [P, NPAIR * DT * P], F16, kind="ExternalInput")
    wv16 = nc.dram_tensor("wv16", [P, NPAIR * DT * P], F16, kind="ExternalInput")
    wq = nc.dram_tensor("wq", [P, NPAIR * DT * P], F8, kind="ExternalInput")
    wk = nc.dram_tensor("wk", [P, NPAIR * DT * P], F8, kind="ExternalInput")
    wv = nc.dram_tensor("wv", [P, NPAIR * DT * P], F8, kind="ExternalInput")
    wo_t = nc.dram_tensor("wo_t", [HPC * DK, D], F16, kind="ExternalInput")
    # q|k|v biases in one transfer
    bqkv = nc.dram_tensor("bqkv", [P, 3 * NPAIR], F32, kind="ExternalInput")
    out = nc.dram_tensor("out_part", [D, S], F16, kind="ExternalOutput")
    dbg = {}
    if debug:
        for nm, shp, dt_ in (
            ("dbg_qt", [P, S], F16),
            ("dbg_kt", [P, S], F16),
            ("dbg_vp", [P, 2 * TT * VW], F16),
            ("dbg_ot", [P, S], F16),
        ):
            dbg[nm] = nc.dram_tensor(nm, shp, dt_, kind="ExternalOutput")

    with TileContext(nc) as tc:
        from contextlib import ExitStack

        with ExitStack() as ctx:
            pool = lambda *a, **k: ctx.enter_context(tc.tile_pool(*a, **k))
            const_pool = pool(name="const", bufs=1)
            xt_pool = pool(name="xt", bufs=1)
            xt16_pool = pool(name="xt16", bufs=1)
            wgt_pool = pool(name="wgt", bufs=3 * NPAIR)
            wo_pool = pool(name="wo", bufs=NPAIR)
            qt_pool = pool(name="qt", bufs=2)
            kt_pool = pool(name="kt", bufs=2)
            vp_pool = pool(name="vp", bufs=2)
            vstg_pool = pool(name="vstg", bufs=2)
            wt_pool = pool(name="wt", bufs=3)
            ot_pool = pool(name="ot", bufs=NPAIR)
            rcs_pool = pool(name="rcs", bufs=3)
            ost_pool = pool(name="ost", bufs=4)
            ps_sc = pool(name="ps_sc", bufs=2, space="PSUM")
            ps_pa = pool(name="ps_pa", bufs=2, space="PSUM")
            ps_pv = pool(name="ps_pv", bufs=2, space="PSUM")

            # --- constants ---
            ident = const_pool.tile([P, P], F16)
            make_identity(nc, ident[:])
            # mask_ut[r, c] = 1 if c >= r else 0 (causal boundary block)
            mask_ut = const_pool.tile([P, P], F16)
            make_upper_triangular(nc, mask_ut[:], val=1.0, diag=True)

            # --- resident inputs, in first-use order: the pair-0 block-0
            # fp16 inputs gate the very first matmul chain, so they go
            # first and everything else queues behind them ---
            def load_wgt16(srcw, p, name):
                t = wgt_pool.tile([P, DT, P], F16, tag="wgt", name=name)
                nc.sync.dma_start(
                    t[:], srcw[:, p * DT * P : (p + 1) * DT * P]
                )
                return t

            boot_t = xt16_pool.tile([P, DT, P + XE], F16, tag="boot", name="boot")
            nc.sync.dma_start(boot_t[:], boot[:])
            wtiles16 = [{"q": None}]  # pair-0 Q comes from boot_t
            xt8j0 = xt16_pool.tile([P, DT, SB], F8, tag="xt8j0", name="xt8j0")
            nc.sync.dma_start(xt8j0[:], xT8j0[:])
            # fp8 weights: pair-0 slice first (gates the first DoubleRow
            # chains), remaining pairs later; per-pair [d, c] blocks stay
            # contiguous so DoubleRow LDWEIGHTS reads 256B runs (FWL)
            w8_all = {}
            w8_src = {"q": wq, "k": wk, "v": wv}
            for nm in ("q", "k", "v"):
                w8_all[nm] = wgt_pool.tile(
                    [P, NPAIR, DT, P], F8, tag="wgt", name=f"w8{nm}"
                )
            nc.sync.dma_start(w8_all["q"][:, 0, :, :], wq[:, : DT * P])
            wtiles16[0]["k"] = load_wgt16(wk16, 0, "wk16_0")
            nc.sync.dma_start(w8_all["k"][:, 0, :, :], wk[:, : DT * P])
            wtiles16[0]["v"] = load_wgt16(wv16, 0, "wv16_0")
            nc.sync.dma_start(w8_all["v"][:, 0, :, :], wv[:, : DT * P])
            bqkv_t = const_pool.tile([P, 3 * NPAIR], F32)
            nc.sync.dma_start(bqkv_t[:], bqkv[:])
            boff = {"q": 0, "k": NPAIR, "v": 2 * NPAIR}
            xt = xt_pool.tile([P, DT, S], F8, tag="xt", name="xt")
            for dh in range(2):
                cs = slice(dh * (DT // 2), (dh + 1) * (DT // 2))
                nc.sync.dma_start(
                    xt[:, cs, :], xT[:, dh * DT * S // 2 : (dh + 1) * DT * S // 2]
                )
            for nm in ("q", "k", "v"):
                nc.sync.dma_start(
                    w8_all[nm][:, 1:NPAIR, :, :], w8_src[nm][:, DT * P :]
                )
            for p in range(1, NPAIR):
                wtiles16.append(
                    {
                        nm: load_wgt16(srcw, p, f"w{nm}16_{p}")
                        for nm, srcw in (
                            ("q", wq16),
                            ("k", wk16),
                            ("v", wv16),
                        )
                    }
                )
            wo_tiles = []
            for p in range(NPAIR):
                t = wo_pool.tile([P, D], F16, tag="wo", name=f"wo{p}")
                nc.sync.dma_start(t[:], wo_t[p * P : (p + 1) * P, :])
                wo_tiles.append(t)

            # V' double buffers with the ones columns pre-set (the V copies
            # only ever touch columns 0..63 of each [tile, head] slot).
            # vp16 holds fp16 copies of tiles 0..3 for the strip-0 AV.
            vp_bufs, vp16_bufs = [], []
            for vb in range(2):
                t = vp_pool.tile([P, 2 * TT * VW], F8, tag="vp", name=f"vp{vb}")
                v4 = t[:].rearrange("r (h i c) -> r h i c", h=2, i=TT)
                t16 = vp_pool.tile(
                    [P, 2 * 4 * VW], F16, tag="vp16", name=f"vp16_{vb}"
                )
                v416 = t16[:].rearrange("r (h i c) -> r h i c", h=2, i=4)
                for h in range(2):
                    nc.vector.memset(v4[:, h, :, DK:VW], 1.0)
                    nc.vector.memset(v416[:, h, :, DK:VW], 1.0)
                vp_bufs.append(t)
                vp16_bufs.append(t16)

            ot_tiles = []
            for p in range(NPAIR):
                wts16 = wtiles16[p]
                qt = qt_pool.tile([P, S], F16, tag="qt")
                kt = kt_pool.tile([P, S], F16, tag="kt")
                vp = vp_bufs[p % 2]
                v4 = vp[:].rearrange("r (h i c) -> r h i c", h=2, i=TT)
                v416 = vp16_bufs[p % 2][:].rearrange(
                    "r (h i c) -> r h i c", h=2, i=4
                )
                ot = ot_pool.tile([P, S], F16, tag="ot")
                ot_tiles.append(ot)

                def proj(nm, j, ps):
                    # first XE columns in fp16 (accuracy for short-context
                    # rows), the rest as fp8 DoubleRow chains (half the PE
                    # passes); block 0 reads the early-arriving x copies
                    if j == 0:
                        for d in range(DT):
                            lhs16 = (
                                boot_t[:, d, 0:P]
                                if (p == 0 and nm == "q")
                                else wts16[nm][:, d, :]
                            )
                            nc.tensor.matmul(
                                ps[:, 0:XE],
                                lhs16,
                                boot_t[:, d, P : P + XE],
                                start=(d == 0),
                                stop=(d == DT - 1),
                            )
                        for dd in range(0, DT, 2):
                            nc.tensor.matmul(
                                ps[:, XE:SB],
                                w8_all[nm][:, p, dd : dd + 2, :],
                                xt8j0[:, dd : dd + 2, XE:SB],
                                start=(dd == 0),
                                stop=(dd == DT - 2),
                                perf_mode=DR,
                            )
                    else:
                        ss = slice(j * SB, (j + 1) * SB)
                        for dd in range(0, DT, 2):
                            nc.tensor.matmul(
                                ps[:],
                                w8_all[nm][:, p, dd : dd + 2, :],
                                xt[:, dd : dd + 2, ss],
                                start=(dd == 0),
                                stop=(dd == DT - 2),
                                perf_mode=DR,
                            )

                for j in range(NSB):
                    ss = slice(j * SB, (j + 1) * SB)
                    # --- Q/K projections (transposed [dk_pair, s] layout) ---
                    for nm, dest in (("q", qt), ("k", kt)):
                        ps = ps_pv.tile([P, SB], F32, tag="pv", name="ps_p")
                        proj(nm, j, ps)
                        nc.vector.tensor_scalar_add(
                            dest[:, ss],
                            ps[:],
                            bqkv_t[:, boff[nm] + p : boff[nm] + p + 1],
                        )

                    # --- V projection + PE transpose to natural [t, dk] ---
                    ps = ps_pv.tile([P, SB], F32, tag="pv", name="ps_v")
                    proj("v", j, ps)
                    vst = vstg_pool.tile([P, SB], F16, tag="vstg")
                    nc.vector.tensor_scalar_add(
                        vst[:],
                        ps[:],
                        bqkv_t[:, boff["v"] + p : boff["v"] + p + 1],
                    )
                    for u in range(SB // P):
                        tg = (SB // P) * j + u
                        pt = ps_pv.tile([P, P], F16, tag="pv", name="pt")
                        nc.tensor.transpose(
                            pt[:], vst[:, u * P : (u + 1) * P], ident[:]
                        )
                        # pt cols 0:64 = head0 V rows, 64:128 = head1; one
                        # strided copy drops both into V' (fp16 -> fp8)
                        nc.vector.tensor_copy(
                            v4[:, :, tg, 0:DK],
                            pt[:].rearrange("r (h c) -> r h c", h=2),
                        )
                        if j == 0:
                            nc.vector.tensor_copy(
                                v416[:, :, tg, 0:DK],
                                pt[:].rearrange("r (h c) -> r h c", h=2),
                            )

                    # --- causal attention strip j ---
                    pa0 = ps_pa.tile([P, SB], F32, tag="pa", name="pa0")
                    pa1 = ps_pa.tile([P, SB], F32, tag="pa", name="pa1")
                    nt = 4 * j + 4
                    wtp = None
                    for i in range(nt):
                        r = i - 4 * j
                        c0 = P * max(r, 0)
                        # both heads' scores into one PSUM tile: disjoint PE
                        # row groups and disjoint PSUM banks -> concurrent
                        sc = ps_sc.tile([P, 2 * SB], F32, tag="sc", name="sc")
                        nc.tensor.matmul(
                            sc[:, c0:SB],
                            kt[0:DK, i * P : (i + 1) * P],
                            qt[0:DK, j * SB + c0 : (j + 1) * SB],
                            start=True,
                            stop=True,
                        )
                        nc.tensor.matmul(
                            sc[:, SB + c0 : 2 * SB],
                            kt[DK:P, i * P : (i + 1) * P],
                            qt[DK:P, j * SB + c0 : (j + 1) * SB],
                            start=True,
                            stop=True,
                        )
                        if i % 2 == 0:
                            wtp = wt_pool.tile(
                                [P, 2 * 2 * SB], F16 if j == 0 else F8, tag="wt"
                            )
                        w4 = wtp[:].rearrange(
                            "r (t h c) -> r t h c", t=2, h=2
                        )
                        # one exp for both heads, trimmed to >= the causal
                        # frontier (strided AP over the two 512-col halves)
                        nc.scalar.activation(
                            w4[:, i % 2, :, c0:SB],
                            sc[:].rearrange("r (h c) -> r h c", h=2)[
                                :, :, c0:SB
                            ],
                            AF.Exp,
                            scale=SCL,
                        )
                        if r >= 0:
                            for h in range(2):
                                # on GpSimd: DVE is the busier engine and
                                # its backlog was stalling the V-transposes
                                nc.gpsimd.tensor_tensor(
                                    w4[:, i % 2, h, c0 : c0 + P],
                                    w4[:, i % 2, h, c0 : c0 + P],
                                    mask_ut[:],
                                    MUL,
                                )
                            if j == 0:
                                # strip 0: single fp16 AV matmul per tile
                                for h, pa in ((0, pa0), (1, pa1)):
                                    nc.tensor.matmul(
                                        pa[:, c0:],
                                        v416[:, h, i, :],
                                        w4[:, i % 2, h, c0:SB],
                                        start=(i == 0),
                                        stop=(i == nt - 1),
                                    )
                            elif r % 2 == 0:
                                # even diagonal tile: only its solo 128-col
                                # strip (the rest rides the next tile's DR)
                                for h, pa in ((0, pa0), (1, pa1)):
                                    nc.tensor.matmul(
                                        pa[:, c0 : c0 + P],
                                        v4[:, h, i, :],
                                        w4[:, i % 2, h, c0 : c0 + P],
                                        start=False,
                                        stop=False,
                                    )
                            else:
                                # odd diagonal tile: DoubleRow over the
                                # columns where both tiles are causally live
                                for h, pa in ((0, pa0), (1, pa1)):
                                    nc.tensor.matmul(
                                        pa[:, c0:],
                                        v4[:, h, i - 1 : i + 1, :],
                                        w4[:, :, h, c0:SB],
                                        start=False,
                                        stop=(i == nt - 1),
                                        perf_mode=DR,
                                    )
                        elif i % 2 == 1:
                            # two full t-tiles: DoubleRow AV (K=256)
                            for h, pa in ((0, pa0), (1, pa1)):
                                nc.tensor.matmul(
                                    pa[:],
                                    v4[:, h, i - 1 : i + 1, :],
                                    w4[:, :, h, :],
                                    start=(i == 1),
                                    stop=False,
                                    perf_mode=DR,
                                )
                    # normalize by the softmax denominator, which the
                    # ones-block of V' replicated into PSUM rows 64..127.
                    # (copy to SBUF first: reciprocal_approx_fast is a custom
                    # DVE op and cannot read PSUM)
                    den = rcs_pool.tile([P, SB], F32, tag="den", name="den")
                    nc.vector.tensor_copy(den[0:DK, :], pa0[DK:P, :])
                    nc.vector.tensor_copy(den[DK:P, :], pa1[DK:P, :])
                    rcs = rcs_pool.tile([P, SB], F32, tag="rcs", name="rcs")
                    nc.vector.reciprocal_approx_fast(rcs[:], den[:])
                    nc.vector.tensor_tensor(
                        ot[0:DK, ss], pa0[0:DK, :], rcs[0:DK, :], MUL
                    )
                    nc.vector.tensor_tensor(
                        ot[DK:P, ss], pa1[0:DK, :], rcs[DK:P, :], MUL
                    )
                if debug and p == 0:
                    nc.sync.dma_start(dbg["dbg_qt"][:], qt[:])
                    nc.sync.dma_start(dbg["dbg_kt"][:], kt[:])
                    vps = const_pool.tile([P, 2 * TT * VW], F16, name="vps")
                    nc.vector.tensor_copy(vps[:], vp[:])
                    nc.sync.dma_start(dbg["dbg_vp"][:], vps[:])
                    nc.sync.dma_start(dbg["dbg_ot"][:], ot[:])

            # --- output projection: j-major so block j overlaps the last
            # pair's attention on later blocks; accumulate the 4 pairs ---
            # (NB: tc.high_priority() here regressed — unbounded by 23us,
            # a bounded offset=500 on j=0 by ~3us.  Priority hoisting puts
            # sem-waits earlier in the runtime FIFO queues than their
            # dependencies resolve.)
            for j in range(NSB):
                ss = slice(j * SB, (j + 1) * SB)
                for m in range(DT):
                    ps = ps_pv.tile([P, SB], F32, tag="pv", name="ps_o")
                    for p in range(NPAIR):
                        nc.tensor.matmul(
                            ps[:],
                            wo_tiles[p][:, m * P : (m + 1) * P],
                            ot_tiles[p][:, ss],
                            start=(p == 0),
                            stop=(p == NPAIR - 1),
                        )
                    st = ost_pool.tile([P, SB], F16, tag="ost")
                    if j >= 2:
                        # late sweeps run after the last exp: ScalarE is idle
                        # there, so drain PSUM on it in parallel with DVE
                        nc.scalar.activation(
                            st[:], ps[:], AF.Copy, scale=1.0 / WSCALE
                        )
                    else:
                        nc.vector.tensor_scalar_mul(st[:], ps[:], 1.0 / WSCALE)
                    nc.sync.dma_start(out[m * P : (m + 1) * P, ss], st[:])

    nc.compile()
    return nc


_NC_CACHE = None


def _get_nc():
    global _NC_CACHE
    if _NC_CACHE is None:
        _NC_CACHE = build_nc()
    return _NC_CACHE


def _f8(a):
    return np.asarray(a, dtype=np.float32).astype(ml_dtypes.float8_e4m3fn)


def _core_inputs(x, Wq, bq, Wk, bk, Wv, bv, Wo, c):
    b, g = c // 2, c % 2
    hs = slice(g * HPC, (g + 1) * HPC)
    heads = range(g * HPC, (g + 1) * HPC)
    def _xlay(a):
        # [D, ncol] -> [r, (d c)]
        return np.ascontiguousarray(
            a.reshape(DT, P, -1).transpose(1, 0, 2).reshape(P, -1)
        )

    def _wlay(a):
        # [D, HPC*DK] -> [r, (p d c)]
        return np.ascontiguousarray(
            a.reshape(DT, P, NPAIR, P).transpose(1, 2, 0, 3).reshape(P, -1)
        )

    xTf = np.ascontiguousarray(x[b].T)
    xT = _f8(_xlay(xTf))
    xT8j0 = _f8(_xlay(xTf[:, :SB]))
    wq_f = _wlay(WSCALE * np.concatenate([Wq[h] for h in heads], axis=1))
    wk_f = _wlay(WSCALE * np.concatenate([Wk[h] for h in heads], axis=1))
    wv_f = _wlay(WSCALE * np.concatenate([Wv[h] for h in heads], axis=1))
    wq_c, wk_c, wv_c = _f8(wq_f), _f8(wk_f), _f8(wv_f)
    boot_c = np.ascontiguousarray(
        np.concatenate(
            [
                wq_f[:, : DT * P].astype(np.float16).reshape(P, DT, P),
                _xlay(xTf[:, :128]).astype(np.float16).reshape(P, DT, 128),
            ],
            axis=2,
        ).reshape(P, -1)
    )
    def _blay(b_):
        return WSCALE * np.concatenate([b_[h] for h in heads]).reshape(NPAIR, P).T

    bqkv_c = np.ascontiguousarray(
        np.concatenate([_blay(bq), _blay(bk), _blay(bv)], axis=1),
        dtype=np.float32,
    )
    wo_c = np.ascontiguousarray(
        Wo[:, g * HPC * DK : (g + 1) * HPC * DK].T, dtype=np.float16
    )
    return {
        "xT": xT,
        "boot": boot_c,
        "xT8j0": xT8j0,
        "wq": wq_c,
        "wk": wk_c,
        "wv": wv_c,
        "wq16": wq_f.astype(np.float16),
        "wk16": wk_f.astype(np.float16),
        "wv16": wv_f.astype(np.float16),
        "wo_t": wo_c,
        "bqkv": bqkv_c,
    }


def kernel(x, Wq, bq, Wk, bk, Wv, bv, Wo, bo, _trace=False, _tmpdir=None):
    x = np.asarray(x, dtype=np.float32)
    nc = _get_nc()
    in_maps = [
        _core_inputs(x, Wq, bq, Wk, bk, Wv, bv, Wo, c) for c in range(NCORES)
    ]
    kw = {}
    if _trace:
        kw = dict(trace=True, tmpdir=_tmpdir)
    res = bass_utils.run_bass_kernel_spmd(
        nc, in_maps, core_ids=list(range(NCORES)), **kw
    )
    bo = np.asarray(bo, dtype=np.float32)
    out = np.empty((B, S, D), dtype=np.float32)
    for b in range(B):
        part = res.results[2 * b]["out_part"].astype(np.float32) + res.results[
            2 * b + 1
        ]["out_part"].astype(np.float32)
        out[b] = part.T + bo
    if _trace:
        kernel._last_results = res
    return out
